# revision 17
# baseline (speedup 1.0000x reference)
"""ConvDecoder Bass kernel for Trainium2, SPMD over 8 NeuronCores.

Math (per batch element b, one per core):
    r_conv = Conv1d(r, conv_w, SAME) + conv_b            # (C, N_IN)
    d[n,m] = (xc[n] - xt[m])^2                           # (N_IN, N_OUT)
    wt_c   = exp(-0.5 * d / exp(sigma_c)^2)
    z[m,c] = sum_n r_conv[c,n] * wt_c[n,m]
    out    = z @ lin_w.T + lin_b                         # (N_OUT, OUT_C)

v3 (single length-scale fast path):
  - All inputs arrive in 3 packed DMAs: pA fp32 (xc per-partition, lin_b
    column, xt broadcast to 128 partitions for both m-halves) and pB bf16
    (host-built im2col stack incl. ones bias row, conv weights, lin128).
  - All matmuls run in bf16 (single pass instead of fp32's LOW+HIGH
    double pass). E-chunk intermediates (diff, dsq) are fp16; E itself
    bf16. xc/xt stay fp32 where it matters for exp-argument accuracy.
  - Conv1d as 4 im2col matmuls (81,128)^T @ (81,16); results land in a
    zero-padded (128, 4*32) bf16 lhsT whose 32-row strips feed the RBF
    reduction.
  - Per m-half: 4 E chunks (sub+sq on DVE/ACT/GpSimd round-robin, exp on
    ACT), 4 strip matmuls into one PSUM tile via tile_position, one
    PSUM->bf16 copy, then ONE output matmul lhsT=lin128 producing
    y^T (32, 512), bias-added and stored with a single DMA. The host
    transposes y^T back. (The 128-row contraction folds the 4 n-tile
    partials and the channel reduction into the output matmul.)
  - Multi-group sigma falls back to the proven v2 kernel below.
"""

import numpy as np
import ml_dtypes

import concourse.bass as bass
import concourse.mybir as mybir
from concourse.tile import TileContext
from concourse.bass_utils import run_bass_kernel_spmd

F32 = mybir.dt.float32
F16 = mybir.dt.float16
BF16 = mybir.dt.bfloat16

B, N_IN, N_OUT, C, OUT_C, KW = 8, 512, 1024, 16, 32, 5
N_CORES = 8
NT = N_IN // 128   # n tiles (4)
MH = N_OUT // 512  # m halves (2)
MT = 512 // 128    # m tiles per half (4)

# packed pA column offsets
XC0, LB0, XT0 = 0, 4, 5
PA_W = XT0 + N_OUT          # 1029
# pB: im2col stack (cols 0:512) + lin-folded conv weights (cols 512:544)
PB_W = N_IN + OUT_C         # 544
WA0 = N_IN

# per-chunk sub+square engine: 'dve' (vector) or 'act' (scalar Square
# w/ per-partition bias) — balanced against ACT's 4 exp passes.
# (gpsimd tensor_scalar is a ~7.5us ucode path that also starves DVE's
# SBUF access: never put elementwise work there.)
MODES = ("dve", "act", "dve", "dve")
ACT_K = MODES.index("act")


# --- walrus workaround -----------------------------------------------------
# This container's walrus accepts at most ONE semaphore wait per TPB
# instruction, but Tile's scheduler attaches several (joins + tail drain).
# Hoist all but the last wait of each instruction onto fresh wait-only
# EventSemaphore instructions inserted right before it on the same engine.
_ws_ctr = [0]


def _split_multi_waits(nc):
    for fn in nc.m.functions:
        for blk in fn.blocks:
            insts = blk.instructions
            if not any(
                ins.sync_info and len(ins.sync_info.on_wait) > 1 for ins in insts
            ):
                continue
            out = []
            for ins in insts:
                si = ins.sync_info
                waits = list(si.on_wait) if si else []
                if len(waits) > 1:
                    for w in waits[:-1]:
                        _ws_ctr[0] += 1
                        ev = mybir.InstEventSemaphore(
                            name=f"waitsplit_{_ws_ctr[0]}", ins=[], outs=[]
                        )
                        ev.engine = ins.engine
                        ev.sync_info = mybir.SyncInfo(on_wait=[w], on_update=[])
                        nc.register_instruction(ev)
                        out.append(ev)
                    ins.sync_info = mybir.SyncInfo(
                        on_wait=[waits[-1]], on_update=list(si.on_update)
                    )
                out.append(ins)
            insts[:] = out


# --- v3 single-group kernel build ------------------------------------------
def _build_fast(a):
    nc = bass.Bass()
    pa_in = nc.dram_tensor("pA", [128, PA_W], F32, kind="ExternalInput")
    pb_in = nc.dram_tensor("pB", [C * KW + 1, PB_W], BF16, kind="ExternalInput")
    yt_out = nc.dram_tensor("yt", [OUT_C, N_OUT], F32, kind="ExternalOutput")

    Exp = mybir.ActivationFunctionType.Exp
    Square = mybir.ActivationFunctionType.Square

    with TileContext(nc) as tc:
        with (
            tc.tile_pool(name="const", bufs=1) as cpool,
            tc.tile_pool(name="work", bufs=1) as wpool,
            tc.tile_pool(name="psum", bufs=1, space="PSUM") as ppool,
        ):
            # packed inputs across three engine DMA queues; the 16 DMA
            # engines are a shared ~150GB/s pool, so order matters: pb
            # (gates conv) and A1 (xc+xt0, gates half 0) stream first;
            # A2 (xt1) issues from scalar after the warm exp and lands
            # just in time for half 1.
            pb = cpool.tile([C * KW + 1, PB_W], BF16)
            nc.gpsimd.dma_start(out=pb[:], in_=pb_in[:])
            pa = cpool.tile([128, PA_W], F32)
            nc.sync.dma_start(out=pa[:, 0 : XT0 + 512], in_=pa_in[:, 0 : XT0 + 512])

            # dummy exp on a memset tile: hoists the ~1.3us ACT table load
            # to t~=0 with no data dependency
            warm = cpool.tile([128, 1], F32)
            nc.vector.memset(warm[:], 0.0)
            warmo = cpool.tile([128, 1], F32)
            nc.scalar.activation(warmo[:], warm[:], Exp)
            nc.scalar.dma_start(out=pa[:, XT0 + 512 :], in_=pa_in[:, XT0 + 512 :])

            xc_pt = pa[:, XC0 : XC0 + NT]
            xtb = pa[:, XT0 : XT0 + N_OUT]
            linb = pa[0:OUT_C, LB0 : LB0 + 1]

            # ---- conv matmuls, linear layer folded into the weights ----
            # cps2[n, o] = sum_p stack[p, n] * wa2[p, o] directly yields
            # R2^T = (lin_w @ (conv(r)+b))^T per n-tile: the z/linear
            # stages collapse into the RBF accumulation below.
            cps = ppool.tile([128, NT * OUT_C], F32, tag="smallps", bufs=1)
            for t in range(NT):
                nc.tensor.matmul(
                    cps[:, t * OUT_C : (t + 1) * OUT_C],
                    lhsT=pb[0 : C * KW + 1, t * 128 : (t + 1) * 128],
                    rhs=pb[0 : C * KW + 1, WA0 : WA0 + OUT_C],
                    start=True,
                    stop=True,
                )
            rsb = cpool.tile([128, NT * OUT_C], BF16)

            # ---- E chunks + accumulating output matmuls, per m-half ----
            # Half 0 needs only A1 (xc+xt0, lands first); half 1's xt1
            # arrives on the scalar-issued DMA just in time. y^T[o, m]
            # accumulates over the 4 n-tiles directly in PSUM; the two
            # halves sit on different PE column groups (tile_position
            # column == PSUM base partition) and overlap at the seam.
            yps_t = [
                ppool.tile([OUT_C, 512], F32, tag="yps", bufs=2,
                           name=f"yps{mh}")
                for mh in range(MH)
            ]
            for mh in range(MH):
                xtb_h = pa[:, XT0 + mh * 512 : XT0 + (mh + 1) * 512]
                dsq_t = {}
                # (xc - xt)^2 == (xt - xc)^2: scale=-1 with bias=+xc
                # needs no negated-xc tile
                dsq = wpool.tile([128, 512], F16, name=f"dsq{mh}_{ACT_K}")
                nc.scalar.activation(dsq[:], xtb_h, Square, scale=-1.0,
                                     bias=xc_pt[:, ACT_K : ACT_K + 1])
                dsq_t[ACT_K] = dsq
                for k in range(NT):
                    if MODES[k] == "act":
                        continue
                    diff = wpool.tile([128, 512], F16, name=f"diff{mh}_{k}")
                    nc.vector.tensor_scalar(
                        diff[:], xtb_h, xc_pt[:, k : k + 1], None,
                        op0=mybir.AluOpType.subtract,
                    )
                    dsq = wpool.tile([128, 512], F16, name=f"dsq{mh}_{k}")
                    nc.vector.tensor_mul(out=dsq[:], in0=diff[:], in1=diff[:])
                    dsq_t[k] = dsq
                    if mh == 0 and k == 2:
                        # conv PSUM -> bf16 lhsT: slotted late enough
                        # that DVE never stalls on the conv matmuls, but
                        # before the first output matmul needs it
                        nc.vector.tensor_copy(out=rsb[:], in_=cps[:])
                for k in range(NT):
                    esb = wpool.tile([128, 512], BF16, name=f"e{mh}_{k}")
                    nc.scalar.activation(esb[:], dsq_t[k][:], Exp,
                                         scale=-float(a))
                    nc.tensor.matmul(
                        yps_t[mh][:],
                        lhsT=rsb[:, k * OUT_C : (k + 1) * OUT_C],
                        rhs=esb[:],
                        start=(k == 0),
                        stop=(k == NT - 1),
                    )

            # ---- bias-add + store, the two halves on parallel engines ----
            osb0 = wpool.tile([OUT_C, 512], F32, name="o0")
            nc.vector.tensor_scalar(
                osb0[:], yps_t[0][:], linb, None,
                op0=mybir.AluOpType.add,
            )
            nc.sync.dma_start(out=yt_out[:, 0:512], in_=osb0[:])
            # half 1 ends the kernel: bias-add and store in two parallel
            # column halves (DVE+ACT, then gpsimd+sync) to shorten the
            # final chain
            osb1a = wpool.tile([OUT_C, 256], F32, name="o1a")
            nc.vector.tensor_scalar(
                osb1a[:], yps_t[1][:, 0:256], linb, None,
                op0=mybir.AluOpType.add,
            )
            osb1b = wpool.tile([OUT_C, 256], F32, name="o1b")
            nc.scalar.activation(
                osb1b[:], yps_t[1][:, 256:512],
                mybir.ActivationFunctionType.Identity, bias=linb,
            )
            nc.gpsimd.dma_start(out=yt_out[:, 512:768], in_=osb1a[:])
            nc.sync.dma_start(out=yt_out[:, 768:1024], in_=osb1b[:])

    _split_multi_waits(nc)
    return nc


# --- v2 general fallback (multi length-scale groups) -----------------------
def _build_general(groups):
    """groups: tuple of (c0, c1, a) with contiguous channel ranges."""
    nc = bass.Bass()
    r_in = nc.dram_tensor("r", [C, N_IN], F32, kind="ExternalInput")
    xc_in = nc.dram_tensor("xc", [1, N_IN], F32, kind="ExternalInput")
    xt_in = nc.dram_tensor("xt", [1, N_OUT], F32, kind="ExternalInput")
    wconv = nc.dram_tensor("w_aug", [C * KW + 1, C], F32, kind="ExternalInput")
    wlin = nc.dram_tensor("lin128", [128, OUT_C], F32, kind="ExternalInput")
    blin = nc.dram_tensor("lin_b", [1, OUT_C], F32, kind="ExternalInput")
    y_out = nc.dram_tensor("y", [N_OUT, OUT_C], F32, kind="ExternalOutput")

    Exp = mybir.ActivationFunctionType.Exp

    with TileContext(nc) as tc:
        with (
            tc.tile_pool(name="const", bufs=1) as cpool,
            tc.tile_pool(name="work", bufs=1) as wpool,
            tc.tile_pool(name="psum", bufs=1, space="PSUM") as ppool,
        ):
            xc_pt = cpool.tile([128, NT], F32)
            nc.sync.dma_start(
                out=xc_pt[:], in_=xc_in[0, :].rearrange("(t p) -> p t", p=128)
            )
            xtb = []
            for mh in range(MH):
                t = cpool.tile([128, 512], F32, name=f"xtb{mh}")
                nc.sync.dma_start(
                    out=t[:],
                    in_=xt_in[0:1, mh * 512 : (mh + 1) * 512].partition_broadcast(128),
                )
                xtb.append(t)
            warm = cpool.tile([128, NT], F32)
            nc.scalar.activation(warm[:], xc_pt[:], Exp)

            wa = cpool.tile([C * KW + 1, C], F32)
            nc.gpsimd.dma_start(out=wa[:], in_=wconv[:])
            wl = cpool.tile([128, OUT_C], F32)
            nc.gpsimd.dma_start(out=wl[:], in_=wlin[:])
            blb = cpool.tile([128, OUT_C], F32)
            nc.gpsimd.dma_start(out=blb[:], in_=blin[0:1, :].partition_broadcast(128))

            stack = cpool.tile([C * KW + 1, N_IN], F32)
            nc.vector.memset(stack[:, :], 0.0)
            pad = KW // 2
            for k in range(KW):
                lo = max(0, pad - k)
                hi = min(N_IN, N_IN + pad - k)
                eng = nc.gpsimd if k % 2 else nc.sync
                eng.dma_start(
                    out=stack[1 + C * k : 1 + C * (k + 1), lo:hi],
                    in_=r_in[:, lo + k - pad : hi + k - pad],
                )
            nc.vector.memset(stack[0:1, :], 1.0)

            r_t = []
            for t in range(NT):
                cps = ppool.tile([128, C], F32, tag="smallps", bufs=2,
                                 name=f"cps{t}")
                nc.tensor.matmul(
                    cps[:],
                    lhsT=stack[:, t * 128 : (t + 1) * 128],
                    rhs=wa[:],
                    start=True,
                    stop=True,
                )
                rsb = cpool.tile([128, 2 * C], F32, name=f"rsb{t}")
                nc.vector.memset(rsb[:, C : 2 * C], 0.0)
                nc.vector.tensor_copy(out=rsb[:, 0:C], in_=cps[:])
                r_t.append(rsb)

            for mh in range(MH):
                z_sb = wpool.tile([C, 512], F32, tag="zsb", bufs=2,
                                  name=f"z{mh}")
                for gi, (c0, c1, ag) in enumerate(groups):
                    gsz = c1 - c0
                    zps = ppool.tile([gsz, 512], F32, tag="zps", bufs=2,
                                     name=f"zps{mh}_{gi}")
                    for k in range(NT):
                        diff = wpool.tile([128, 512], F32, tag="diff",
                                          bufs=3, name=f"df{mh}_{gi}_{k}")
                        nc.vector.tensor_scalar(
                            diff[:], xtb[mh][:], xc_pt[:, k : k + 1], None,
                            op0=mybir.AluOpType.subtract,
                        )
                        dsq = wpool.tile([128, 512], F32, tag="dsq",
                                         bufs=3, name=f"dq{mh}_{gi}_{k}")
                        nc.vector.tensor_mul(out=dsq[:], in0=diff[:],
                                             in1=diff[:])
                        esb = wpool.tile([128, 512], F32, tag="esb",
                                         bufs=3, name=f"e{mh}_{gi}_{k}")
                        nc.scalar.activation(esb[:], dsq[:], Exp,
                                             scale=-float(ag))
                        nc.tensor.matmul(
                            zps[:],
                            lhsT=r_t[k][:, c0:c1],
                            rhs=esb[:],
                            start=(k == 0),
                            stop=(k == NT - 1),
                        )
                    if c0 % 32 == 0:
                        nc.vector.tensor_copy(out=z_sb[c0:c1, :], in_=zps[:])
                    else:
                        nc.sync.dma_start(out=z_sb[c0:c1, :], in_=zps[:])

                for mt in range(MT):
                    ops = ppool.tile([128, OUT_C], F32, tag="smallps", bufs=2,
                                     name=f"ops{mh}_{mt}")
                    nc.tensor.matmul(
                        ops[:],
                        lhsT=z_sb[:, mt * 128 : (mt + 1) * 128],
                        rhs=wl[0:C, :],
                        start=True,
                        stop=True,
                    )
                    osb = wpool.tile([128, OUT_C], F32, tag="osb", bufs=3,
                                     name=f"o{mh}_{mt}")
                    nc.vector.tensor_add(out=osb[:], in0=ops[:], in1=blb[:])
                    m0 = mh * 512 + mt * 128
                    nc.sync.dma_start(out=y_out[m0 : m0 + 128, :], in_=osb[:])

    _split_multi_waits(nc)
    return nc


_cache = {}


def _get_nc(key, builder, *args):
    if key not in _cache:
        _cache[key] = builder(*args)
    return _cache[key]


def _groups_of(sigma):
    scales = np.exp(np.asarray(sigma, np.float64))
    a = 0.5 / scales**2
    perm = np.argsort(a, kind="stable")
    a_s = a[perm]
    groups = []
    c0 = 0
    for c in range(1, C + 1):
        if c == C or a_s[c] != a_s[c0]:
            groups.append((c0, c, float(a_s[c0])))
            c0 = c
    return tuple(groups), perm


def _lin128_of(lin_w, perm):
    lin_w_t = np.asarray(lin_w, np.float32).T[perm]
    lin128 = np.zeros((128, OUT_C), np.float32)
    for j in range(4):
        lin128[32 * j : 32 * j + C] = lin_w_t
    return lin128


def _prepare_fast(a, r, x_context, x_target, conv_w, conv_b, lin_w, lin_b):
    r = np.asarray(r, np.float32)
    xc = np.asarray(x_context, np.float32).reshape(B, N_IN)
    xt = np.asarray(x_target, np.float32).reshape(B, N_OUT)
    w_aug = np.concatenate(
        [np.asarray(conv_b, np.float64)[None, :],
         np.asarray(conv_w, np.float64).transpose(2, 1, 0).reshape(C * KW, C)],
        axis=0,
    )
    # fold the pointwise linear into the conv weights (parameter-only):
    # wa2[p, o] = sum_c w_aug[p, c] * lin_w[o, c]
    wa2 = (w_aug @ np.asarray(lin_w, np.float64).T).astype(np.float32)

    pb = np.zeros((C * KW + 1, PB_W), np.float32)
    pb[0 : C * KW + 1, WA0 : WA0 + OUT_C] = wa2
    pb[0, 0:N_IN] = 1.0

    in_maps = []
    for b in range(B):
        pa = np.zeros((128, PA_W), np.float32)
        pa[:, XC0 : XC0 + NT] = xc[b].reshape(NT, 128).T
        pa[0:OUT_C, LB0] = np.asarray(lin_b, np.float32)
        pa[:, XT0 : XT0 + N_OUT] = xt[b][None, :]
        pbb = pb.copy()
        rpad = np.zeros((C, N_IN + KW - 1), np.float32)
        rpad[:, KW // 2 : KW // 2 + N_IN] = r[b]
        # stack row 1+16k+c, col j = r[c, j + k - 2]
        win = np.lib.stride_tricks.sliding_window_view(rpad, N_IN, axis=1)
        pbb[1 : 1 + C * KW, 0:N_IN] = (
            win.transpose(1, 0, 2).reshape(C * KW, N_IN)
        )
        in_maps.append(
            {
                "pA": np.ascontiguousarray(pa),
                "pB": np.ascontiguousarray(pbb, dtype=ml_dtypes.bfloat16),
            }
        )
    return in_maps


def _prepare_general(groups, perm, r, x_context, x_target, conv_w, conv_b,
                     lin_w, lin_b):
    r = np.asarray(r, np.float32)
    x_context = np.asarray(x_context, np.float32)
    x_target = np.asarray(x_target, np.float32)
    w_aug = np.concatenate(
        [np.asarray(conv_b, np.float32)[None, :],
         np.asarray(conv_w, np.float32).transpose(2, 1, 0).reshape(C * KW, C)],
        axis=0,
    )[:, perm]
    w_aug = np.ascontiguousarray(w_aug, np.float32)
    lin128 = _lin128_of(lin_w, perm)
    lin_b_row = np.ascontiguousarray(
        np.asarray(lin_b, np.float32)[None, :], np.float32
    )
    return [
        {
            "r": np.ascontiguousarray(r[b]),
            "xc": np.ascontiguousarray(x_context[b].reshape(1, N_IN)),
            "xt": np.ascontiguousarray(x_target[b].reshape(1, N_OUT)),
            "w_aug": w_aug,
            "lin128": lin128,
            "lin_b": lin_b_row,
        }
        for b in range(B)
    ]


def kernel(**inputs):
    sigma = inputs["sigma"]
    groups, perm = _groups_of(sigma)
    if len(groups) == 1:
        a = groups[0][2]
        in_maps = _prepare_fast(
            a, inputs["r"], inputs["x_context"], inputs["x_target"],
            inputs["conv_w"], inputs["conv_b"], inputs["lin_w"],
            inputs["lin_b"],
        )
        nc = _get_nc(("fast", np.float32(a).tobytes()), _build_fast, a)
        res = run_bass_kernel_spmd(nc, in_maps, list(range(N_CORES)))
        return np.ascontiguousarray(
            np.stack([res.results[b]["yt"].T for b in range(B)], axis=0)
        )
    in_maps = _prepare_general(
        groups, perm, inputs["r"], inputs["x_context"], inputs["x_target"],
        inputs["conv_w"], inputs["conv_b"], inputs["lin_w"], inputs["lin_b"],
    )
    key = ("gen",) + tuple(
        (c0, c1, np.float32(a).tobytes()) for c0, c1, a in groups
    )
    nc = _get_nc(key, _build_general, groups)
    res = run_bass_kernel_spmd(nc, in_maps, list(range(N_CORES)))
    return np.stack([res.results[b]["y"] for b in range(B)], axis=0)


# revision 18
# speedup vs baseline: 1.0522x; 1.0522x over previous
"""ConvDecoder Bass kernel for Trainium2, SPMD over 8 NeuronCores.

Math (per batch element b, one per core):
    r_conv = Conv1d(r, conv_w, SAME) + conv_b            # (C, N_IN)
    d[n,m] = (xc[n] - xt[m])^2                           # (N_IN, N_OUT)
    wt_c   = exp(-0.5 * d / exp(sigma_c)^2)
    z[m,c] = sum_n r_conv[c,n] * wt_c[n,m]
    out    = z @ lin_w.T + lin_b                         # (N_OUT, OUT_C)

v3 (single length-scale fast path):
  - All inputs arrive in 3 packed DMAs: pA fp32 (xc per-partition, lin_b
    column, xt broadcast to 128 partitions for both m-halves) and pB bf16
    (host-built im2col stack incl. ones bias row, conv weights, lin128).
  - All matmuls run in bf16 (single pass instead of fp32's LOW+HIGH
    double pass). E-chunk intermediates (diff, dsq) are fp16; E itself
    bf16. xc/xt stay fp32 where it matters for exp-argument accuracy.
  - Conv1d as 4 im2col matmuls (81,128)^T @ (81,16); results land in a
    zero-padded (128, 4*32) bf16 lhsT whose 32-row strips feed the RBF
    reduction.
  - Per m-half: 4 E chunks (sub+sq on DVE/ACT/GpSimd round-robin, exp on
    ACT), 4 strip matmuls into one PSUM tile via tile_position, one
    PSUM->bf16 copy, then ONE output matmul lhsT=lin128 producing
    y^T (32, 512), bias-added and stored with a single DMA. The host
    transposes y^T back. (The 128-row contraction folds the 4 n-tile
    partials and the channel reduction into the output matmul.)
  - Multi-group sigma falls back to the proven v2 kernel below.
"""

import numpy as np
import ml_dtypes

import concourse.bass as bass
import concourse.mybir as mybir
from concourse.tile import TileContext
from concourse.bass_utils import run_bass_kernel_spmd

F32 = mybir.dt.float32
F16 = mybir.dt.float16
BF16 = mybir.dt.bfloat16

B, N_IN, N_OUT, C, OUT_C, KW = 8, 512, 1024, 16, 32, 5
N_CORES = 8
NT = N_IN // 128   # n tiles (4)
MH = N_OUT // 512  # m halves (2)
MT = 512 // 128    # m tiles per half (4)

# packed pA column offsets
XC0, LB0, XT0 = 0, 4, 5
PA_W = XT0 + N_OUT          # 1029
# pB: im2col stack (cols 0:512) + lin-folded conv weights (cols 512:544)
PB_W = N_IN + OUT_C         # 544
WA0 = N_IN

# per-chunk sub+square engine: 'dve' (vector) or 'act' (scalar Square
# w/ per-partition bias) — balanced against ACT's 4 exp passes.
# (gpsimd tensor_scalar is a ~7.5us ucode path that also starves DVE's
# SBUF access: never put elementwise work there.)
MODES = ("dve", "act", "dve", "dve")
ACT_K = MODES.index("act")


# --- walrus workaround -----------------------------------------------------
# This container's walrus accepts at most ONE semaphore wait per TPB
# instruction, but Tile's scheduler attaches several (joins + tail drain).
# Hoist all but the last wait of each instruction onto fresh wait-only
# EventSemaphore instructions inserted right before it on the same engine.
_ws_ctr = [0]


def _split_multi_waits(nc):
    for fn in nc.m.functions:
        for blk in fn.blocks:
            insts = blk.instructions
            if not any(
                ins.sync_info and len(ins.sync_info.on_wait) > 1 for ins in insts
            ):
                continue
            out = []
            for ins in insts:
                si = ins.sync_info
                waits = list(si.on_wait) if si else []
                if len(waits) > 1:
                    for w in waits[:-1]:
                        _ws_ctr[0] += 1
                        ev = mybir.InstEventSemaphore(
                            name=f"waitsplit_{_ws_ctr[0]}", ins=[], outs=[]
                        )
                        ev.engine = ins.engine
                        ev.sync_info = mybir.SyncInfo(on_wait=[w], on_update=[])
                        nc.register_instruction(ev)
                        out.append(ev)
                    ins.sync_info = mybir.SyncInfo(
                        on_wait=[waits[-1]], on_update=list(si.on_update)
                    )
                out.append(ins)
            insts[:] = out


# --- v3 single-group kernel build ------------------------------------------
def _build_fast(a):
    nc = bass.Bass()
    pa_in = nc.dram_tensor("pA", [128, PA_W], F32, kind="ExternalInput")
    pb_in = nc.dram_tensor("pB", [C * KW + 1, PB_W], BF16, kind="ExternalInput")
    yt_out = nc.dram_tensor("yt", [OUT_C, N_OUT], F32, kind="ExternalOutput")

    Exp = mybir.ActivationFunctionType.Exp
    Square = mybir.ActivationFunctionType.Square

    with TileContext(nc) as tc:
        with (
            tc.tile_pool(name="const", bufs=1) as cpool,
            tc.tile_pool(name="work", bufs=1) as wpool,
            tc.tile_pool(name="psum", bufs=1, space="PSUM") as ppool,
        ):
            # packed inputs across three engine DMA queues; the 16 DMA
            # engines are a shared ~150GB/s pool, so order matters: pb
            # (gates conv) and A1 (xc+xt0, gates half 0) stream first;
            # A2 (xt1) issues from scalar after the warm exp and lands
            # just in time for half 1.
            pb = cpool.tile([C * KW + 1, PB_W], BF16)
            nc.gpsimd.dma_start(out=pb[:], in_=pb_in[:])
            pa = cpool.tile([128, PA_W], F32)
            nc.sync.dma_start(out=pa[:, 0 : XT0 + 512], in_=pa_in[:, 0 : XT0 + 512])

            # dummy exp on a memset tile: hoists the ~1.3us ACT table load
            # to t~=0 with no data dependency
            warm = cpool.tile([128, 1], F32)
            nc.vector.memset(warm[:], 0.0)
            warmo = cpool.tile([128, 1], F32)
            nc.scalar.activation(warmo[:], warm[:], Exp)
            nc.scalar.dma_start(out=pa[:, XT0 + 512 :], in_=pa_in[:, XT0 + 512 :])

            xc_pt = pa[:, XC0 : XC0 + NT]
            xtb = pa[:, XT0 : XT0 + N_OUT]
            linb = pa[0:OUT_C, LB0 : LB0 + 1]

            # ---- conv matmuls, linear layer folded into the weights ----
            # cps2[n, o] = sum_p stack[p, n] * wa2[p, o] directly yields
            # R2^T = (lin_w @ (conv(r)+b))^T per n-tile: the z/linear
            # stages collapse into the RBF accumulation below.
            cps = ppool.tile([128, NT * OUT_C], F32, tag="smallps", bufs=1)
            for t in range(NT):
                nc.tensor.matmul(
                    cps[:, t * OUT_C : (t + 1) * OUT_C],
                    lhsT=pb[0 : C * KW + 1, t * 128 : (t + 1) * 128],
                    rhs=pb[0 : C * KW + 1, WA0 : WA0 + OUT_C],
                    start=True,
                    stop=True,
                )
            rsb = cpool.tile([128, NT * OUT_C], BF16)

            # ---- E chunks + accumulating output matmuls, per m-half ----
            # Half 0 needs only A1 (xc+xt0, lands first); half 1's xt1
            # arrives on the scalar-issued DMA just in time. y^T[o, m]
            # accumulates over the 4 n-tiles directly in PSUM; the two
            # halves sit on different PE column groups (tile_position
            # column == PSUM base partition) and overlap at the seam.
            yps_t = [
                ppool.tile([OUT_C, 512], F32, tag="yps", bufs=2,
                           name=f"yps{mh}")
                for mh in range(MH)
            ]
            for mh in range(MH):
                xtb_h = pa[:, XT0 + mh * 512 : XT0 + (mh + 1) * 512]
                dsq_t = {}
                # (xc - xt)^2 == (xt - xc)^2: scale=-1 with bias=+xc
                # needs no negated-xc tile
                dsq = wpool.tile([128, 512], F16, name=f"dsq{mh}_{ACT_K}")
                nc.scalar.activation(dsq[:], xtb_h, Square, scale=-1.0,
                                     bias=xc_pt[:, ACT_K : ACT_K + 1])
                dsq_t[ACT_K] = dsq
                for k in range(NT):
                    if MODES[k] == "act":
                        continue
                    diff = wpool.tile([128, 512], F16, name=f"diff{mh}_{k}")
                    nc.vector.tensor_scalar(
                        diff[:], xtb_h, xc_pt[:, k : k + 1], None,
                        op0=mybir.AluOpType.subtract,
                    )
                    dsq = wpool.tile([128, 512], F16, name=f"dsq{mh}_{k}")
                    nc.vector.tensor_mul(out=dsq[:], in0=diff[:], in1=diff[:])
                    dsq_t[k] = dsq
                    if mh == 0 and k == 2:
                        # conv PSUM -> bf16 lhsT: slotted late enough
                        # that DVE never stalls on the conv matmuls, but
                        # before the first output matmul needs it
                        nc.vector.tensor_copy(out=rsb[:], in_=cps[:])
                for k in range(NT):
                    esb = wpool.tile([128, 512], BF16, name=f"e{mh}_{k}")
                    nc.scalar.activation(esb[:], dsq_t[k][:], Exp,
                                         scale=-float(a))
                    nc.tensor.matmul(
                        yps_t[mh][:],
                        lhsT=rsb[:, k * OUT_C : (k + 1) * OUT_C],
                        rhs=esb[:],
                        start=(k == 0),
                        stop=(k == NT - 1),
                    )

            # ---- bias-add + store, the two halves on parallel engines ----
            osb0 = wpool.tile([OUT_C, 512], F32, name="o0")
            nc.vector.tensor_scalar(
                osb0[:], yps_t[0][:], linb, None,
                op0=mybir.AluOpType.add,
            )
            nc.sync.dma_start(out=yt_out[:, 0:512], in_=osb0[:])
            # half 1 ends the kernel: bias-add and store in two parallel
            # column halves (DVE+ACT, then gpsimd+sync) to shorten the
            # final chain
            osb1 = wpool.tile([OUT_C, 512], F32, name="o1")
            nc.vector.tensor_scalar(
                osb1[:, 0:256], yps_t[1][:, 0:256], linb, None,
                op0=mybir.AluOpType.add,
            )
            nc.scalar.activation(
                osb1[:, 256:512], yps_t[1][:, 256:512],
                mybir.ActivationFunctionType.Identity, bias=linb,
            )
            nc.gpsimd.dma_start(out=yt_out[:, 512:768], in_=osb1[:, 0:256])
            nc.sync.dma_start(out=yt_out[:, 768:1024], in_=osb1[:, 256:512])

    _split_multi_waits(nc)
    return nc


# --- v2 general fallback (multi length-scale groups) -----------------------
def _build_general(groups):
    """groups: tuple of (c0, c1, a) with contiguous channel ranges."""
    nc = bass.Bass()
    r_in = nc.dram_tensor("r", [C, N_IN], F32, kind="ExternalInput")
    xc_in = nc.dram_tensor("xc", [1, N_IN], F32, kind="ExternalInput")
    xt_in = nc.dram_tensor("xt", [1, N_OUT], F32, kind="ExternalInput")
    wconv = nc.dram_tensor("w_aug", [C * KW + 1, C], F32, kind="ExternalInput")
    wlin = nc.dram_tensor("lin128", [128, OUT_C], F32, kind="ExternalInput")
    blin = nc.dram_tensor("lin_b", [1, OUT_C], F32, kind="ExternalInput")
    y_out = nc.dram_tensor("y", [N_OUT, OUT_C], F32, kind="ExternalOutput")

    Exp = mybir.ActivationFunctionType.Exp

    with TileContext(nc) as tc:
        with (
            tc.tile_pool(name="const", bufs=1) as cpool,
            tc.tile_pool(name="work", bufs=1) as wpool,
            tc.tile_pool(name="psum", bufs=1, space="PSUM") as ppool,
        ):
            xc_pt = cpool.tile([128, NT], F32)
            nc.sync.dma_start(
                out=xc_pt[:], in_=xc_in[0, :].rearrange("(t p) -> p t", p=128)
            )
            xtb = []
            for mh in range(MH):
                t = cpool.tile([128, 512], F32, name=f"xtb{mh}")
                nc.sync.dma_start(
                    out=t[:],
                    in_=xt_in[0:1, mh * 512 : (mh + 1) * 512].partition_broadcast(128),
                )
                xtb.append(t)
            warm = cpool.tile([128, NT], F32)
            nc.scalar.activation(warm[:], xc_pt[:], Exp)

            wa = cpool.tile([C * KW + 1, C], F32)
            nc.gpsimd.dma_start(out=wa[:], in_=wconv[:])
            wl = cpool.tile([128, OUT_C], F32)
            nc.gpsimd.dma_start(out=wl[:], in_=wlin[:])
            blb = cpool.tile([128, OUT_C], F32)
            nc.gpsimd.dma_start(out=blb[:], in_=blin[0:1, :].partition_broadcast(128))

            stack = cpool.tile([C * KW + 1, N_IN], F32)
            nc.vector.memset(stack[:, :], 0.0)
            pad = KW // 2
            for k in range(KW):
                lo = max(0, pad - k)
                hi = min(N_IN, N_IN + pad - k)
                eng = nc.gpsimd if k % 2 else nc.sync
                eng.dma_start(
                    out=stack[1 + C * k : 1 + C * (k + 1), lo:hi],
                    in_=r_in[:, lo + k - pad : hi + k - pad],
                )
            nc.vector.memset(stack[0:1, :], 1.0)

            r_t = []
            for t in range(NT):
                cps = ppool.tile([128, C], F32, tag="smallps", bufs=2,
                                 name=f"cps{t}")
                nc.tensor.matmul(
                    cps[:],
                    lhsT=stack[:, t * 128 : (t + 1) * 128],
                    rhs=wa[:],
                    start=True,
                    stop=True,
                )
                rsb = cpool.tile([128, 2 * C], F32, name=f"rsb{t}")
                nc.vector.memset(rsb[:, C : 2 * C], 0.0)
                nc.vector.tensor_copy(out=rsb[:, 0:C], in_=cps[:])
                r_t.append(rsb)

            for mh in range(MH):
                z_sb = wpool.tile([C, 512], F32, tag="zsb", bufs=2,
                                  name=f"z{mh}")
                for gi, (c0, c1, ag) in enumerate(groups):
                    gsz = c1 - c0
                    zps = ppool.tile([gsz, 512], F32, tag="zps", bufs=2,
                                     name=f"zps{mh}_{gi}")
                    for k in range(NT):
                        diff = wpool.tile([128, 512], F32, tag="diff",
                                          bufs=3, name=f"df{mh}_{gi}_{k}")
                        nc.vector.tensor_scalar(
                            diff[:], xtb[mh][:], xc_pt[:, k : k + 1], None,
                            op0=mybir.AluOpType.subtract,
                        )
                        dsq = wpool.tile([128, 512], F32, tag="dsq",
                                         bufs=3, name=f"dq{mh}_{gi}_{k}")
                        nc.vector.tensor_mul(out=dsq[:], in0=diff[:],
                                             in1=diff[:])
                        esb = wpool.tile([128, 512], F32, tag="esb",
                                         bufs=3, name=f"e{mh}_{gi}_{k}")
                        nc.scalar.activation(esb[:], dsq[:], Exp,
                                             scale=-float(ag))
                        nc.tensor.matmul(
                            zps[:],
                            lhsT=r_t[k][:, c0:c1],
                            rhs=esb[:],
                            start=(k == 0),
                            stop=(k == NT - 1),
                        )
                    if c0 % 32 == 0:
                        nc.vector.tensor_copy(out=z_sb[c0:c1, :], in_=zps[:])
                    else:
                        nc.sync.dma_start(out=z_sb[c0:c1, :], in_=zps[:])

                for mt in range(MT):
                    ops = ppool.tile([128, OUT_C], F32, tag="smallps", bufs=2,
                                     name=f"ops{mh}_{mt}")
                    nc.tensor.matmul(
                        ops[:],
                        lhsT=z_sb[:, mt * 128 : (mt + 1) * 128],
                        rhs=wl[0:C, :],
                        start=True,
                        stop=True,
                    )
                    osb = wpool.tile([128, OUT_C], F32, tag="osb", bufs=3,
                                     name=f"o{mh}_{mt}")
                    nc.vector.tensor_add(out=osb[:], in0=ops[:], in1=blb[:])
                    m0 = mh * 512 + mt * 128
                    nc.sync.dma_start(out=y_out[m0 : m0 + 128, :], in_=osb[:])

    _split_multi_waits(nc)
    return nc


_cache = {}


def _get_nc(key, builder, *args):
    if key not in _cache:
        _cache[key] = builder(*args)
    return _cache[key]


def _groups_of(sigma):
    scales = np.exp(np.asarray(sigma, np.float64))
    a = 0.5 / scales**2
    perm = np.argsort(a, kind="stable")
    a_s = a[perm]
    groups = []
    c0 = 0
    for c in range(1, C + 1):
        if c == C or a_s[c] != a_s[c0]:
            groups.append((c0, c, float(a_s[c0])))
            c0 = c
    return tuple(groups), perm


def _lin128_of(lin_w, perm):
    lin_w_t = np.asarray(lin_w, np.float32).T[perm]
    lin128 = np.zeros((128, OUT_C), np.float32)
    for j in range(4):
        lin128[32 * j : 32 * j + C] = lin_w_t
    return lin128


def _prepare_fast(a, r, x_context, x_target, conv_w, conv_b, lin_w, lin_b):
    r = np.asarray(r, np.float32)
    xc = np.asarray(x_context, np.float32).reshape(B, N_IN)
    xt = np.asarray(x_target, np.float32).reshape(B, N_OUT)
    w_aug = np.concatenate(
        [np.asarray(conv_b, np.float64)[None, :],
         np.asarray(conv_w, np.float64).transpose(2, 1, 0).reshape(C * KW, C)],
        axis=0,
    )
    # fold the pointwise linear into the conv weights (parameter-only):
    # wa2[p, o] = sum_c w_aug[p, c] * lin_w[o, c]
    wa2 = (w_aug @ np.asarray(lin_w, np.float64).T).astype(np.float32)

    pb = np.zeros((C * KW + 1, PB_W), np.float32)
    pb[0 : C * KW + 1, WA0 : WA0 + OUT_C] = wa2
    pb[0, 0:N_IN] = 1.0

    in_maps = []
    for b in range(B):
        pa = np.zeros((128, PA_W), np.float32)
        pa[:, XC0 : XC0 + NT] = xc[b].reshape(NT, 128).T
        pa[0:OUT_C, LB0] = np.asarray(lin_b, np.float32)
        pa[:, XT0 : XT0 + N_OUT] = xt[b][None, :]
        pbb = pb.copy()
        rpad = np.zeros((C, N_IN + KW - 1), np.float32)
        rpad[:, KW // 2 : KW // 2 + N_IN] = r[b]
        # stack row 1+16k+c, col j = r[c, j + k - 2]
        win = np.lib.stride_tricks.sliding_window_view(rpad, N_IN, axis=1)
        pbb[1 : 1 + C * KW, 0:N_IN] = (
            win.transpose(1, 0, 2).reshape(C * KW, N_IN)
        )
        in_maps.append(
            {
                "pA": np.ascontiguousarray(pa),
                "pB": np.ascontiguousarray(pbb, dtype=ml_dtypes.bfloat16),
            }
        )
    return in_maps


def _prepare_general(groups, perm, r, x_context, x_target, conv_w, conv_b,
                     lin_w, lin_b):
    r = np.asarray(r, np.float32)
    x_context = np.asarray(x_context, np.float32)
    x_target = np.asarray(x_target, np.float32)
    w_aug = np.concatenate(
        [np.asarray(conv_b, np.float32)[None, :],
         np.asarray(conv_w, np.float32).transpose(2, 1, 0).reshape(C * KW, C)],
        axis=0,
    )[:, perm]
    w_aug = np.ascontiguousarray(w_aug, np.float32)
    lin128 = _lin128_of(lin_w, perm)
    lin_b_row = np.ascontiguousarray(
        np.asarray(lin_b, np.float32)[None, :], np.float32
    )
    return [
        {
            "r": np.ascontiguousarray(r[b]),
            "xc": np.ascontiguousarray(x_context[b].reshape(1, N_IN)),
            "xt": np.ascontiguousarray(x_target[b].reshape(1, N_OUT)),
            "w_aug": w_aug,
            "lin128": lin128,
            "lin_b": lin_b_row,
        }
        for b in range(B)
    ]


def kernel(**inputs):
    sigma = inputs["sigma"]
    groups, perm = _groups_of(sigma)
    if len(groups) == 1:
        a = groups[0][2]
        in_maps = _prepare_fast(
            a, inputs["r"], inputs["x_context"], inputs["x_target"],
            inputs["conv_w"], inputs["conv_b"], inputs["lin_w"],
            inputs["lin_b"],
        )
        nc = _get_nc(("fast", np.float32(a).tobytes()), _build_fast, a)
        res = run_bass_kernel_spmd(nc, in_maps, list(range(N_CORES)))
        return np.ascontiguousarray(
            np.stack([res.results[b]["yt"].T for b in range(B)], axis=0)
        )
    in_maps = _prepare_general(
        groups, perm, inputs["r"], inputs["x_context"], inputs["x_target"],
        inputs["conv_w"], inputs["conv_b"], inputs["lin_w"], inputs["lin_b"],
    )
    key = ("gen",) + tuple(
        (c0, c1, np.float32(a).tobytes()) for c0, c1, a in groups
    )
    nc = _get_nc(key, _build_general, groups)
    res = run_bass_kernel_spmd(nc, in_maps, list(range(N_CORES)))
    return np.stack([res.results[b]["y"] for b in range(B)], axis=0)
